# revision 75
# baseline (speedup 1.0000x reference)
"""Trainium2 Bass kernel for nn_Attention_69861938037646.

Math (per batch b):
  c[b]      = last[b]@W2 + avg[b]@W3                             [32]
  proj[s,f] = x[b,s,:]@W1 + c[b]                                 [200,32]
  scores[s] = sigmoid(proj[s,:] + b123) @ W4                     [200]
  out[b]    = sum_s scores[s] * x[b,s,:]                         [32]

Device layout (per core, 512 batches as 4 streams x 128 batches):
  x4[32j+e, 200g+s] = x[b0+128j+g, s, e]   (bf16, [128, 25600])
Per 800-col chunk (4 batches):
  - PE: bias matmul (cgT rows stationary, fp8 0/1 indicator moving) +
    block-diag W1 matmul accumulate into PSUM proj, then block-diag
    replicated-W4 matmul broadcasts scores to every e-partition
  - ACT: one sigmoid instruction, with b123 folded into its per-partition
    bias operand (ACT only ever runs Sigmoid; table warmed at t=0)
  - DVE: one custom fused multiply+prefix-sum op (MULSCAN, registered at
    import) over x*scores; per-batch sums are the differences of the scan
    at the 4 segment boundaries (tiny subtract on idle GpSimd)
Pipeline: proj and scores in separate double-buffered PSUM pools (4x2
banks); W4+reduce emitted two chunks behind ind/W1 so the in-order PE
queue never stalls on sigmoid; x pairs flood the serial DMA bus from the
Pool/SWDGE queue while the indicator table streams in 8 just-in-time
windows; output streamed out in 4 pieces.
Steady state alternates between the PE floor (3 x 800 moving columns
at 2.4GHz = 999ns/chunk) and the DVE scan floor (958ns); TimelineSim
43.6us/core vs the 84.1us baseline.
"""

import sys
from contextlib import ExitStack

import numpy as np
import ml_dtypes

sys.path.insert(0, "/opt/trn_rl_repo")

import concourse.bass as bass  # noqa: E402
import concourse.tile as tile  # noqa: E402
from concourse import bacc, mybir  # noqa: E402
from concourse.bass_utils import run_bass_kernel_spmd  # noqa: E402
from concourse import dve_ops as _dve_ops  # noqa: E402
from concourse.dve_spec import (  # noqa: E402
    AluOp as _DveAluOp,
    Spec as _DveSpec,
    Src0 as _Src0,
    Src1 as _Src1,
    _has_src1,
    lower as _dve_lower,
    scan as _dve_scan,
)
from concourse.dve_uop import DveOpSpec as _DveOpSpec  # noqa: E402


def _register_mulscan():
    """Register a fused multiply + prefix-sum custom DVE op:
    out[p, k] = sum_{t<=k} in0[p, t] * in1[p, t]. Per-batch segment sums are
    recovered by differencing the scan at segment boundaries."""
    name = "MULSCAN_ANT"
    if name in _dve_ops._SUB_OPCODE_FOR_NAME:
        return _dve_ops.CUSTOM_DVE_SPECS[name] and next(
            op for op in _dve_ops.OPS if op.name == name
        )

    def ref(in0, in1, s0, s1, imm2):
        p = in0.shape[0]
        prod = in0.astype(np.float32).reshape(p, -1) * in1.astype(np.float32).reshape(
            p, -1
        )
        return np.cumsum(prod, axis=1, dtype=np.float32)

    spec = _DveSpec(body=_dve_scan(_DveAluOp.ADD, _Src0 * _Src1), reference=ref)
    shas = {}
    for ver in ("v3", "v4"):
        s = _DveOpSpec(
            name=name, opcode=0, uops=_dve_lower(spec, ver=ver), rd1_en=_has_src1(spec)
        )
        shas[ver] = s.sha(ver)
    op = _dve_ops.DveOp(name, spec, subdim=False, uops_sha=shas)
    row = max(_dve_ops._SUB_OPCODE_FOR_NAME.values()) + 1
    assert row < 0x20
    _dve_ops.OPS.append(op)
    _dve_ops.CUSTOM_DVE_SPECS[name] = spec
    _dve_ops._SUB_OPCODE_FOR_NAME[name] = row
    return op


_MULSCAN = _register_mulscan()

B, S, E = 4096, 200, 32
NCORES = 8
BPC = B // NCORES          # 512 batches per core
NS = 4                     # partition streams
G = BPC // NS              # 128 batches per stream
COLS = G * S               # 25600 columns per core
CHUNK = 4 * S              # 800 cols (4 batches) per psum chunk
NCHUNK = COLS // CHUNK     # 32 chunks
PAIR = 2 * CHUNK           # 1600-col DMA granularity (DRAM block)

BF16 = mybir.dt.bfloat16
F32 = mybir.dt.float32

_CACHE = {}


def _build_program():
    nc = bacc.Bacc("TRN2", target_bir_lowering=False)
    # pair-major: rows 128p..128p+128 hold DMA pair p, contiguous in DRAM
    x4 = nc.dram_tensor("x4", [128 * (NCHUNK // 2), PAIR], BF16, kind="ExternalInput")
    # bf16 constants in one tensor: w1d|w2d|w3d|w4b|la4
    CB = 4 * 128 + 2 * G
    cb16 = nc.dram_tensor("cb16", [128, CB], BF16, kind="ExternalInput")
    # 0/1 indicator table, exact in fp8 (halves its DMA footprint); shipped
    # as uint8 bytes (PJRT input path lacks fp8) and bitcast on device
    FP8 = mybir.dt.float8e4
    ind8 = nc.dram_tensor("ind8", [128, 32 * S], mybir.dt.uint8, kind="ExternalInput")
    # f32 constants: b123 (applied as the sigmoid's per-partition bias)
    cb32 = nc.dram_tensor("cb32", [128, 129], F32, kind="ExternalInput")
    out4 = nc.dram_tensor("out4", [128, G], F32, kind="ExternalOutput")

    SL = [(0, 512), (512, CHUNK - 512)]

    with tile.TileContext(nc) as tc, ExitStack() as ctx:
        consts = ctx.enter_context(tc.tile_pool(name="consts", bufs=1))

        # warm the ACT sigmoid table immediately (1.3us load, otherwise paid
        # in front of the first real sigmoid)
        warm = consts.tile([128, 1], F32, tag="warm")
        nc.vector.memset(warm[:], 0.0)
        nc.scalar.activation(warm[:], warm[:], mybir.ActivationFunctionType.Sigmoid)

        cb16t = consts.tile([128, CB], BF16, tag="cb16t")
        indt8 = consts.tile([128, 32 * S], mybir.dt.uint8, tag="indt8")
        indt = indt8[:].bitcast(FP8)
        # constants load: small setup part first, then the indicator table in
        # 8 just-in-time windows (window w is first needed by chunk w); x
        # pairs flood the serial DMA bus from the Pool queue concurrently
        nc.sync.dma_start(cb16t[:], cb16[:])
        cb32t = consts.tile([128, 129], F32, tag="cb32t")

        def load_ind_window(w, eng):
            eng.dma_start(
                indt8[:, 800 * w : 800 * w + 800], ind8[:, 800 * w : 800 * w + 800]
            )

        load_ind_window(0, nc.sync)
        nc.sync.dma_start(cb32t[:], cb32[:])
        w1t = cb16t[:, 0:128]
        w2t = cb16t[:, 128:256]
        w3t = cb16t[:, 256:384]
        w4t = cb16t[:, 384:512]
        lat = cb16t[:, 512 : 512 + 2 * G]
        bt = cb32t[:, 0:1]
        idt = cb32t[:, 1:129]

        c4 = consts.tile([128, G], F32, tag="c4")
        cgT = consts.tile([128, 128], BF16, tag="cgT")
        out_sb = consts.tile([128, G], F32, tag="out_sb")
        # prefix-scan scratch, col 0 pre-zeroed so segment sums come from
        # differencing scan[200(g+1)] - scan[200g]
        scans = [
            consts.tile([128, CHUNK + 1], F32, tag=f"scan{j}", name=f"scan{j}")
            for j in range(2)
        ]
        for t in scans:
            nc.vector.memset(t[:, 0:1], 0.0)

        # ---- setup: cgT[g, 32j+f] = (last@W2 + avg@W3)[128j+g, f]
        # (c computed in [f, g] form, then transposed on PE)
        with tc.tile_pool(name="sps", bufs=1, space="PSUM") as sps:
            cps = sps.tile([128, G], F32, tag="cps")
            nc.tensor.matmul(cps[:], w2t[:], lat[:, :G], start=True, stop=False)
            nc.tensor.matmul(cps[:], w3t[:], lat[:, G:], start=False, stop=True)
            # copies on DVE so ACT only ever runs Sigmoid (no act-table thrash)
            nc.vector.tensor_scalar_add(c4[:], cps[:], 0.0)
            tpp = sps.tile([128, 128], F32, tag="tpp")
            nc.tensor.transpose(tpp[:], c4[:], idt[:])
            nc.vector.tensor_scalar_add(cgT[:], tpp[:], 0.0)

        xpool = ctx.enter_context(tc.tile_pool(name="xpool", bufs=8))
        ppool = ctx.enter_context(tc.tile_pool(name="ppool", bufs=2, space="PSUM"))
        spool = ctx.enter_context(tc.tile_pool(name="spool", bufs=2, space="PSUM"))
        sgpool = ctx.enter_context(tc.tile_pool(name="sgpool", bufs=4))

        live = {}
        xpair = [None] * (NCHUNK // 2)

        def head(q):
            if q % 2 == 0:
                # odd indicator windows lead the Pool queue (tiny descgen,
                # needed one chunk after the pair)
                if q < 8:
                    load_ind_window(q + 1, nc.gpsimd)
                xt = xpool.tile([128, PAIR], BF16, tag="xt")
                eng = nc.sync if q == 2 else nc.gpsimd
                eng.dma_start(xt[:], x4[bass.ts(q // 2, 128), :])
                if q == 2:
                    for _w in (2, 4, 6):
                        load_ind_window(_w, nc.sync)
                xpair[q // 2] = xt
            xt = xpair[q // 2]
            xo = (q % 2) * CHUNK
            b32 = 32 * (q // 8)
            io = (q % 8) * CHUNK
            proj = ppool.tile([128, CHUNK], F32, tag="proj")
            for s0, w in SL:
                nc.tensor.matmul(
                    proj[:, s0 : s0 + w],
                    cgT[b32 : b32 + 32, :],
                    indt[b32 : b32 + 32, io + s0 : io + s0 + w],
                    start=True, stop=False,
                    tile_position=(b32, 0),
                    skip_group_check=True,
                )
            for s0, w in SL:
                nc.tensor.matmul(
                    proj[:, s0 : s0 + w],
                    w1t[:],
                    xt[:, xo + s0 : xo + s0 + w],
                    start=False, stop=True,
                    skip_group_check=True,
                )
            sig = sgpool.tile([128, CHUNK], BF16, tag="sig")
            nc.scalar.activation(
                sig[:], proj[:], mybir.ActivationFunctionType.Sigmoid, bias=bt
            )
            live[q] = (xt, sig)

        def tail(q):
            xt, sig = live.pop(q)
            xo = (q % 2) * CHUNK
            scores = spool.tile([128, CHUNK], F32, tag="scores")
            for s0, w in SL:
                nc.tensor.matmul(
                    scores[:, s0 : s0 + w],
                    w4t[:],
                    sig[:, s0 : s0 + w],
                    start=True, stop=True,
                    skip_group_check=True,
                )
            # fused multiply + prefix-sum in one DVE op, then per-batch sums
            # by differencing the scan at the 4 segment boundaries
            sc = scans[q % 2]
            nc.vector._custom_dve(
                _MULSCAN,
                out=sc[:, 1 : CHUNK + 1],
                in0=xt[:, xo : xo + CHUNK],
                in1=scores[:],
            )
            nc.gpsimd.tensor_tensor(
                out=out_sb[:, 4 * q : 4 * q + 4],
                in0=sc[:, S : CHUNK + 1 : S],
                in1=sc[:, 0 : CHUNK + 1 - S : S],
                op=mybir.AluOpType.subtract,
            )

        # tail offset: 1 chunk behind at startup (PE has idle slots anyway),
        # settling to 2 behind so the in-order PE queue never stalls on sigmoid
        done = 0

        def flush_tails(upto):
            nonlocal done
            while done <= upto:
                tail(done)
                # stream finished output columns out every 8 chunks
                if done in (7, 15, 23):
                    o0 = (done - 7) * 4
                    nc.sync.dma_start(
                        out4[:, o0 : o0 + 32], out_sb[:, o0 : o0 + 32]
                    )
                done += 1

        for q in range(NCHUNK):
            head(q)
            if q >= 3:
                flush_tails(q - 2)
            elif q >= 1:
                flush_tails(q - 1)
        flush_tails(NCHUNK - 1)

        nc.sync.dma_start(out4[:, 96:128], out_sb[:, 96:128])

    nc.compile()
    return nc


def _prep_core(all_memory, last_memory, average_memory, i):
    b0 = i * BPC
    xs = np.ascontiguousarray(all_memory[b0 : b0 + BPC])
    x4 = (
        xs.reshape(NS, G, S, E)
        .transpose(0, 3, 1, 2)
        .reshape(128, COLS)
        .astype(ml_dtypes.bfloat16)
    )
    # pair-major DRAM layout: [16*128, 1600], rows 128p..+128 = pair p
    npair = NCHUNK // 2
    x4 = np.ascontiguousarray(
        x4.reshape(128, npair, PAIR).transpose(1, 0, 2).reshape(128 * npair, PAIR)
    )
    la = last_memory[b0 : b0 + BPC].reshape(NS, G, E).transpose(0, 2, 1).reshape(128, G)
    av = (
        average_memory[b0 : b0 + BPC]
        .reshape(NS, G, E)
        .transpose(0, 2, 1)
        .reshape(128, G)
    )
    la4 = np.concatenate([la, av], axis=1).astype(np.float32)
    return {"x4": x4, "la4": la4}


def _shared_inputs(W1, b1, W2, b2, W3, b3, W4):
    def blockdiag(M):
        out = np.zeros((128, 128), ml_dtypes.bfloat16)
        for j in range(NS):
            out[32 * j : 32 * j + 32, 32 * j : 32 * j + 32] = M
        return out

    b123 = (np.asarray(b1) + np.asarray(b2) + np.asarray(b3)).astype(np.float32)
    ind = np.zeros((32, 32 * S), np.float32)
    for r in range(32):
        ind[r, r * S : (r + 1) * S] = 1.0
    cb16 = np.concatenate(
        [
            blockdiag(np.asarray(W1, np.float32)),
            blockdiag(np.asarray(W2, np.float32)),
            blockdiag(np.asarray(W3, np.float32)),
            blockdiag(np.repeat(np.asarray(W4, np.float32).reshape(E, 1), E, 1)),
            np.zeros((128, 2 * G), ml_dtypes.bfloat16),  # la4 slot (per-core)
        ],
        axis=1,
    ).astype(ml_dtypes.bfloat16)
    ind8 = np.tile(ind, (4, 1)).astype(ml_dtypes.float8_e4m3).view(np.uint8)
    cb32 = np.concatenate(
        [
            np.tile(b123.reshape(E, 1), (NS, 1)).astype(np.float32),
            np.eye(128, dtype=np.float32),
        ],
        axis=1,
    ).astype(np.float32)
    return {"cb16": cb16, "ind8": ind8, "cb32": cb32}


def kernel(all_memory, last_memory, average_memory, mask, W1, b1, W2, b2, W3, b3, W4):
    all_memory = np.asarray(all_memory, np.float32)
    last_memory = np.asarray(last_memory, np.float32)
    average_memory = np.asarray(average_memory, np.float32)

    if "nc" not in _CACHE:
        _CACHE["nc"] = _build_program()
    nc = _CACHE["nc"]

    shared = _shared_inputs(W1, b1, W2, b2, W3, b3, W4)
    in_maps = []
    for i in range(NCORES):
        m = _prep_core(all_memory, last_memory, average_memory, i)
        cb16 = shared["cb16"].copy()
        cb16[:, 512 : 512 + 2 * G] = m.pop("la4").astype(ml_dtypes.bfloat16)
        m["cb16"] = cb16
        m["ind8"] = shared["ind8"]
        m["cb32"] = shared["cb32"]
        in_maps.append(m)

    res = run_bass_kernel_spmd(nc, in_maps, list(range(NCORES)))
    outs = []
    for i in range(NCORES):
        o4 = np.asarray(res.results[i]["out4"], np.float32)  # [128, G]
        outs.append(o4.reshape(NS, E, G).transpose(0, 2, 1).reshape(BPC, E))
    return np.concatenate(outs, axis=0).astype(np.float32)
